# revision 13
# baseline (speedup 1.0000x reference)
"""Trainium2 Bass kernel: batch-independent contrastive loss (SupCon-style with
EMA-normalized negatives).

Math (derived from the reference):
  CF = concat(views) [N=4096, D=256], S = CF @ CF.T / T
  Each row i has exactly one positive p(i) = (i+B) mod N; neg_mask keeps the
  diagonal.  With m_i = row max = ||f_i||^2/T:
    Z_i  = sum_j exp(S_ij - m_i)            = e^{-m_i} * P_i,  P_i = sum_j exp(S_ij)
    W_i  = sum_j exp(S_ij - m_i)(S_ij-m_i)  = e^{-m_i} * (Q_i - m_i P_i),
           Q_i = sum_j exp(S_ij) S_ij
    Zneg_i = Z_i - e_pos_i,  Wneg_i = W_i - e_pos_i * Lpos_i
    u_new  = (1-g) u[idx] + g Zneg   (view-0 rows)
    loss_i = Wneg_i / u_new_{i mod B} - Lpos_i ;  output = mean_i loss_i

Sharding: by sample across 8 cores (each core owns 256 samples = 512 anchor
rows covering both views), so the u_new coupling between row b and b+B stays
on-core.  The contrast side (all 4096 columns) is replicated.

v3 design notes:
  - The kernel's irreducible per-element work is one exp (Scalar/ACT is the
    only engine with exp) and one multiply-accumulate for Q = sum E*S (the
    DVE is the only non-ACT engine that can read PSUM; gpsimd rejects both
    PSUM operands and TensorScalarPtr at codegen).  Both engines run ~21us
    for 512x4096 elements/core, so v3 strips EVERYTHING else off them:
    ACT does exactly 16 [128,1024] exp+accum instructions, DVE does exactly
    16 [128,1024] multiply-accumulates plus a ~0.5us tail.
  - Per-row scalars (m, Lpos, e^{-m}, e_pos, and the gamma-folded EMA
    constants) are computed on the host from the f32 features - O(N*D)
    numpy next to the host-side layout/cast prep, vs the N^2*D on-device
    work - and arrive as one tiny [128, 20] aux input.
  - fp8e4(e4m3) DoubleRow matmuls: one matmul folds the full K=256
    contraction at 0.5 cycles/row -> ~4x less PE time than the bf16 chain
    and half the ct DMA bytes, keeping the 4-deep PSUM pipeline always
    ahead of ACT.  fp8 noise only reaches the exp sums (~8e-4 rel on the
    final loss vs the 2e-2 budget).
  - The combine tail runs on Pool (tensor_tensor ops only) except the
    reciprocal and the two ru-dependent final ops (DVE).
  - Steady-state DMAs issue from the SP ring in consumption order; the
    first two ct pieces issue from the ACT ring pre-stream so the first
    matmul's inputs land in parallel with the anchor weights.
"""

import numpy as np
import ml_dtypes

GAMMA = 0.9
TEMP = 0.07
B, V, D = 2048, 2, 256
N = B * V            # 4096 contrast rows/cols
NCORES = 8
SPC = B // NCORES    # 256 samples per core
RPC = V * SPC        # 512 anchor rows per core
RC = RPC // 128      # 4 chunks of 128 anchor rows (0,1: view0; 2,3: view1)
JT = 1024            # contrast-column tile (2 PSUM banks)
NJT = N // JT        # 4
NPC = N // 512       # 8 ct pieces
AUXW = 20            # aux cols: m4[4] lp2[4] em[4] epl[4] emg[2] epu[2]

_CACHE = {}


def _build_module():
    import concourse.bacc as bacc
    import concourse.tile as tile
    from concourse import mybir

    f32 = mybir.dt.float32
    fp8 = mybir.dt.float8e4
    AF = mybir.ActivationFunctionType
    ALU = mybir.AluOpType
    AX = mybir.AxisListType
    DR = mybir.MatmulPerfMode.DoubleRow

    nc = bacc.Bacc(
        "TRN2", target_bir_lowering=False, debug=False, enable_asserts=False
    )
    # anc: [p, k*RPC + r] = cf[row r][k*128+p], fp8
    anc_d = nc.dram_tensor("anc", [128, 2 * RPC], fp8, kind="ExternalInput")
    aux_d = nc.dram_tensor("aux", [128, AUXW], f32, kind="ExternalInput")
    # ct pieces: piece i = contrast cols [i*512,(i+1)*512), [p, k*512+j], fp8
    ct_d = nc.dram_tensor("ct", [NPC, 128, 2 * 512], fp8, kind="ExternalInput")
    out_d = nc.dram_tensor("loss_rows", [128, RC], f32, kind="ExternalOutput")

    with tile.TileContext(nc) as tc:
        with tc.tile_pool(name="singles", bufs=1) as singles, \
             tc.tile_pool(name="psum", bufs=4, space="PSUM") as psum_pool, \
             tc.tile_pool(name="work", bufs=3) as work, \
             tc.tile_pool(name="scr", bufs=2) as scrpool, \
             tc.tile_pool(name="stats", bufs=1) as stats:
            # ---- input DMAs ----
            # head: the rc0 anchor weights (both k-halves) on the SP ring
            # while the ACT ring fetches the first two ct pieces in
            # parallel, so the first matmul fires as early as possible.
            anc_flat = singles.tile([128, 2 * RPC], fp8)
            ct_pc = [None] * NPC
            for i in range(NPC):
                t = singles.tile([128, 2 * 512], fp8, tag=f"ct_{i}")
                ct_pc[i] = t
            nc.sync.dma_start(out=anc_flat[:, 0:128], in_=anc_d[:, 0:128])
            nc.sync.dma_start(out=anc_flat[:, RPC:RPC + 128],
                              in_=anc_d[:, RPC:RPC + 128])
            nc.scalar.dma_start(out=ct_pc[0], in_=ct_d[0])
            nc.scalar.dma_start(out=ct_pc[1], in_=ct_d[1])
            nc.sync.dma_start(out=anc_flat[:, 128:RPC],
                              in_=anc_d[:, 128:RPC])
            nc.sync.dma_start(out=anc_flat[:, RPC + 128:2 * RPC],
                              in_=anc_d[:, RPC + 128:2 * RPC])
            for i in range(2, NPC):
                nc.sync.dma_start(out=ct_pc[i], in_=ct_d[i])
            aux = singles.tile([128, AUXW], f32)
            nc.sync.dma_start(out=aux, in_=aux_d[:, :])
            m4 = aux[:, 0:4]
            lp2 = aux[:, 4:8]
            em = aux[:, 8:12]
            epl = aux[:, 12:16]
            emg = aux[:, 16:18]
            epu = aux[:, 18:20]

            anc_sb = anc_flat.rearrange("p (k r) -> p k r", k=2)

            pacc = stats.tile([128, RC, NJT], f32)
            qacc = stats.tile([128, RC, NJT], f32)

            # ---- main loop: jt-outer so early tiles only need pieces 0-1 ----
            for jt in range(NJT):
                for rc in range(RC):
                    ps = psum_pool.tile([128, JT], f32, tag="ps")
                    for jb in range(JT // 512):
                        pc = ct_pc[jt * (JT // 512) + jb]
                        nc.tensor.matmul(
                            ps[:, jb * 512:(jb + 1) * 512],
                            lhsT=anc_sb[:, :, rc * 128:(rc + 1) * 128],
                            rhs=pc.rearrange("p (k j) -> p k j", k=2),
                            start=True, stop=True,
                            perf_mode=DR,
                        )
                    e_t = work.tile([128, JT], f32, tag="e")
                    nc.scalar.activation(
                        out=e_t, in_=ps, func=AF.Exp, scale=1.0 / TEMP,
                        accum_out=pacc[:, rc, jt:jt + 1],
                    )
                    scr = scrpool.tile([128, JT], f32, tag="qv", name="scr")
                    nc.vector.scalar_tensor_tensor(
                        out=scr, in0=e_t, scalar=1.0 / TEMP,
                        in1=ps, op0=ALU.mult, op1=ALU.mult,
                        accum_out=qacc[:, rc, jt:jt + 1],
                    )

            # ---- combine ----
            # Pool handles the tensor_tensor chain; DVE does the reduces,
            # the reciprocal and the two ru-dependent final ops.
            p4 = stats.tile([128, RC], f32)
            nc.vector.reduce_sum(out=p4, in_=pacc, axis=AX.X)
            # u_new path (view-0 samples): un = g*em*P - g*ep + (1-g)u[idx]
            z2 = stats.tile([128, 2], f32)
            nc.gpsimd.tensor_mul(z2, emg, p4[:, 0:2])
            un = stats.tile([128, 2], f32)
            nc.gpsimd.tensor_add(un, z2, epu)
            ru = stats.tile([128, 2], f32)
            nc.vector.reciprocal(ru, un)

            q4 = stats.tile([128, RC], f32)
            nc.vector.reduce_sum(out=q4, in_=qacc, axis=AX.X)
            t2 = stats.tile([128, RC], f32)
            nc.gpsimd.tensor_mul(t2, m4, p4)
            t3 = stats.tile([128, RC], f32)
            nc.gpsimd.tensor_sub(t3, q4, t2)
            w4 = stats.tile([128, RC], f32)
            nc.gpsimd.tensor_mul(w4, em, t3)
            wn = stats.tile([128, RC], f32)
            nc.gpsimd.tensor_sub(wn, w4, epl)
            c4 = stats.tile([128, RC], f32)
            nc.vector.tensor_mul(c4[:, 0:2], wn[:, 0:2], ru)
            nc.vector.tensor_mul(c4[:, 2:4], wn[:, 2:4], ru)
            out_sb = stats.tile([128, RC], f32)
            nc.vector.tensor_sub(out_sb, c4, lp2)
            nc.sync.dma_start(out=out_d[:, :], in_=out_sb)

    nc.compile()
    return nc


def _get_module():
    if "nc" not in _CACHE:
        _CACHE["nc"] = _build_module()
    return _CACHE["nc"]


def _prep_inputs(index, features, u):
    feats = np.asarray(features, dtype=np.float32)
    idx = np.asarray(index).astype(np.int64).reshape(-1)
    u_np = np.asarray(u, dtype=np.float32).reshape(-1)

    cf = np.ascontiguousarray(feats.transpose(1, 0, 2).reshape(N, D))
    cf8 = cf.astype(ml_dtypes.float8_e4m3)
    ct8 = np.ascontiguousarray(cf8.T)                      # [D, N] fp8
    # [piece, 128, k0-block | k1-block]: piece i = columns [i*512,(i+1)*512)
    ct_in = np.ascontiguousarray(
        ct8.reshape(2, 128, N // 512, 512).transpose(2, 1, 0, 3)
        .reshape(N // 512, 128, 2 * 512))

    # per-row scalars from the f32 features (cheap O(N*D) host work)
    msum = np.einsum('nd,nd->n', cf, cf, dtype=np.float64).astype(np.float32)
    pdot = np.einsum('nd,nd->n', cf[:B], cf[B:],
                     dtype=np.float64).astype(np.float32)   # [B]
    m = msum / TEMP                                         # [N]
    pd4 = np.concatenate([pdot, pdot])                      # [N]
    lp = pd4 / TEMP - m                                     # Lpos [N]
    em = np.exp(-m)
    ep = np.exp(lp)
    epl = ep * lp

    in_maps = []
    for c in range(NCORES):
        rows = np.concatenate([
            np.arange(c * SPC, (c + 1) * SPC),
            np.arange(B + c * SPC, B + (c + 1) * SPC),
        ])
        anc_r = np.ascontiguousarray(ct8[:, rows])         # [128*2(k), RPC]
        anc = np.empty((128, 2 * RPC), dtype=ml_dtypes.float8_e4m3)
        anc[:, 0:RPC] = anc_r[0:128]
        anc[:, RPC:2 * RPC] = anc_r[128:256]

        # aux: [128, rc] layout matches the device's row chunks
        def rcview(v):
            return v[rows].reshape(RC, 128).T              # [128, RC]

        ug_vals = (1.0 - GAMMA) * u_np[idx[c * SPC:(c + 1) * SPC]]
        ug = ug_vals.reshape(2, 128).T                     # [128, 2]
        aux = np.empty((128, AUXW), dtype=np.float32)
        aux[:, 0:4] = rcview(m)
        aux[:, 4:8] = rcview(lp)
        aux[:, 8:12] = rcview(em)
        aux[:, 12:16] = rcview(epl)
        aux[:, 16:18] = GAMMA * rcview(em)[:, 0:2]         # emg (view0)
        aux[:, 18:20] = ug - GAMMA * rcview(ep)[:, 0:2]    # epu (view0)
        in_maps.append({"anc": anc, "aux": aux, "ct": ct_in})
    return in_maps


def _run(in_maps, trace=False, **kw):
    from concourse.bass_utils import run_bass_kernel_spmd

    nc = _get_module()
    return run_bass_kernel_spmd(
        nc, in_maps, core_ids=list(range(NCORES)), trace=trace, **kw
    )


def kernel(index, features, u):
    in_maps = _prep_inputs(index, features, u)
    res = _run(in_maps)
    total = 0.0
    for c in range(NCORES):
        total += np.asarray(res.results[c]["loss_rows"], dtype=np.float64).sum()
    return np.float32(total / N)


# revision 14
# speedup vs baseline: 1.0163x; 1.0163x over previous
"""Trainium2 Bass kernel: batch-independent contrastive loss (SupCon-style with
EMA-normalized negatives).

Math (derived from the reference):
  CF = concat(views) [N=4096, D=256], S = CF @ CF.T / T
  Each row i has exactly one positive p(i) = (i+B) mod N; neg_mask keeps the
  diagonal.  With m_i = row max = ||f_i||^2/T:
    Z_i  = sum_j exp(S_ij - m_i)            = e^{-m_i} * P_i,  P_i = sum_j exp(S_ij)
    W_i  = sum_j exp(S_ij - m_i)(S_ij-m_i)  = e^{-m_i} * (Q_i - m_i P_i),
           Q_i = sum_j exp(S_ij) S_ij
    Zneg_i = Z_i - e_pos_i,  Wneg_i = W_i - e_pos_i * Lpos_i
    u_new  = (1-g) u[idx] + g Zneg   (view-0 rows)
    loss_i = Wneg_i / u_new_{i mod B} - Lpos_i ;  output = mean_i loss_i

Sharding: by sample across 8 cores (each core owns 256 samples = 512 anchor
rows covering both views), so the u_new coupling between row b and b+B stays
on-core.  The contrast side (all 4096 columns) is replicated.

v3 design notes:
  - The kernel's irreducible per-element work is one exp (Scalar/ACT is the
    only engine with exp) and one multiply-accumulate for Q = sum E*S (the
    DVE is the only non-ACT engine that can read PSUM; gpsimd rejects both
    PSUM operands and TensorScalarPtr at codegen).  Both engines run ~21us
    for 512x4096 elements/core, so v3 strips EVERYTHING else off them:
    ACT does exactly 16 [128,1024] exp+accum instructions, DVE does exactly
    16 [128,1024] multiply-accumulates plus a ~0.5us tail.
  - Per-row scalars (m, Lpos, e^{-m}, e_pos, and the gamma-folded EMA
    constants) are computed on the host from the f32 features - O(N*D)
    numpy next to the host-side layout/cast prep, vs the N^2*D on-device
    work - and arrive as one tiny [128, 20] aux input.
  - fp8e4(e4m3) DoubleRow matmuls: one matmul folds the full K=256
    contraction at 0.5 cycles/row -> ~4x less PE time than the bf16 chain
    and half the ct DMA bytes, keeping the 4-deep PSUM pipeline always
    ahead of ACT.  fp8 noise only reaches the exp sums (~8e-4 rel on the
    final loss vs the 2e-2 budget).
  - The combine tail runs on Pool (tensor_tensor ops only) except the
    reciprocal and the two ru-dependent final ops (DVE).
  - Steady-state DMAs issue from the SP ring in consumption order; the
    first two ct pieces issue from the ACT ring pre-stream so the first
    matmul's inputs land in parallel with the anchor weights.
"""

import numpy as np
import ml_dtypes

GAMMA = 0.9
TEMP = 0.07
B, V, D = 2048, 2, 256
N = B * V            # 4096 contrast rows/cols
NCORES = 8
SPC = B // NCORES    # 256 samples per core
RPC = V * SPC        # 512 anchor rows per core
RC = RPC // 128      # 4 chunks of 128 anchor rows (0,1: view0; 2,3: view1)
JT = 1024            # contrast-column tile (2 PSUM banks)
NJT = N // JT        # 4
NPC = N // 512       # 8 ct pieces
AUXW = 20            # aux cols: m4[4] lp2[4] em[4] epl[4] emg[2] epu[2]

_CACHE = {}


def _build_module():
    import concourse.bacc as bacc
    import concourse.tile as tile
    from concourse import mybir

    f32 = mybir.dt.float32
    fp8 = mybir.dt.float8e4
    AF = mybir.ActivationFunctionType
    ALU = mybir.AluOpType
    AX = mybir.AxisListType
    DR = mybir.MatmulPerfMode.DoubleRow

    nc = bacc.Bacc(
        "TRN2", target_bir_lowering=False, debug=False, enable_asserts=False
    )
    # anc: [p, k*RPC + r] = cf[row r][k*128+p], fp8
    anc_d = nc.dram_tensor("anc", [128, 2 * RPC], fp8, kind="ExternalInput")
    aux_d = nc.dram_tensor("aux", [128, AUXW], f32, kind="ExternalInput")
    # ct pieces: piece i = contrast cols [i*512,(i+1)*512), [p, k*512+j], fp8
    ct_d = nc.dram_tensor("ct", [NPC, 128, 2 * 512], fp8, kind="ExternalInput")
    out_d = nc.dram_tensor("loss_rows", [128, RC], f32, kind="ExternalOutput")

    with tile.TileContext(nc) as tc:
        with tc.tile_pool(name="singles", bufs=1) as singles, \
             tc.tile_pool(name="psum", bufs=4, space="PSUM") as psum_pool, \
             tc.tile_pool(name="work", bufs=3) as work, \
             tc.tile_pool(name="scr", bufs=2) as scrpool, \
             tc.tile_pool(name="stats", bufs=1) as stats:
            # ---- input DMAs ----
            # head: the rc0 anchor weights (both k-halves) on the SP ring
            # while the ACT ring fetches the first two ct pieces in
            # parallel, so the first matmul fires as early as possible.
            anc_flat = singles.tile([128, 2 * RPC], fp8)
            ct_pc = [None] * NPC
            for i in range(NPC):
                t = singles.tile([128, 2 * 512], fp8, tag=f"ct_{i}")
                ct_pc[i] = t
            nc.sync.dma_start(out=anc_flat[:, 0:128], in_=anc_d[:, 0:128])
            nc.sync.dma_start(out=anc_flat[:, RPC:RPC + 128],
                              in_=anc_d[:, RPC:RPC + 128])
            nc.scalar.dma_start(out=ct_pc[0], in_=ct_d[0])
            nc.scalar.dma_start(out=ct_pc[1], in_=ct_d[1])
            nc.sync.dma_start(out=anc_flat[:, 128:RPC],
                              in_=anc_d[:, 128:RPC])
            nc.sync.dma_start(out=anc_flat[:, RPC + 128:2 * RPC],
                              in_=anc_d[:, RPC + 128:2 * RPC])
            for i in range(2, NPC):
                nc.sync.dma_start(out=ct_pc[i], in_=ct_d[i])
            aux = singles.tile([128, AUXW], f32)
            nc.sync.dma_start(out=aux, in_=aux_d[:, :])
            m4 = aux[:, 0:4]
            lp2 = aux[:, 4:8]
            em = aux[:, 8:12]
            epl = aux[:, 12:16]
            emg = aux[:, 16:18]
            epu = aux[:, 18:20]

            anc_sb = anc_flat.rearrange("p (k r) -> p k r", k=2)

            # PE warmup: dependency-free fp8 matmuls on a memset tile keep
            # the tensor clock ramping while the inputs stream in.
            warm_sb = singles.tile([128, 1024], fp8)
            nc.vector.memset(warm_sb, 0.0)
            wps = psum_pool.tile([128, JT], f32, tag="ps")
            for w in range(4):
                nc.tensor.matmul(
                    wps[:, 0:512],
                    lhsT=warm_sb.rearrange("p (k r) -> p k r", k=2)[:, :, 0:128],
                    rhs=warm_sb.rearrange("p (k j) -> p k j", k=2),
                    start=True, stop=True, perf_mode=DR,
                )

            pacc = stats.tile([128, RC, NJT], f32)
            qacc = stats.tile([128, RC, NJT], f32)
            qacc2 = stats.tile([128, 1], f32)

            # ---- main loop: jt-outer so early tiles only need pieces 0-1 ----
            for jt in range(NJT):
                for rc in range(RC):
                    ps = psum_pool.tile([128, JT], f32, tag="ps")
                    for jb in range(JT // 512):
                        pc = ct_pc[jt * (JT // 512) + jb]
                        nc.tensor.matmul(
                            ps[:, jb * 512:(jb + 1) * 512],
                            lhsT=anc_sb[:, :, rc * 128:(rc + 1) * 128],
                            rhs=pc.rearrange("p (k j) -> p k j", k=2),
                            start=True, stop=True,
                            perf_mode=DR,
                        )
                    e_t = work.tile([128, JT], f32, tag="e")
                    nc.scalar.activation(
                        out=e_t, in_=ps, func=AF.Exp, scale=1.0 / TEMP,
                        accum_out=pacc[:, rc, jt:jt + 1],
                    )
                    if jt == NJT - 1 and rc == RC - 1:
                        # final tile: two half-width stts shorten the tail
                        scr = scrpool.tile([128, JT], f32, tag="qv", name="scr")
                        nc.vector.scalar_tensor_tensor(
                            out=scr[:, 0:512], in0=e_t[:, 0:512],
                            scalar=1.0 / TEMP, in1=ps[:, 0:512],
                            op0=ALU.mult, op1=ALU.mult,
                            accum_out=qacc[:, rc, jt:jt + 1],
                        )
                        nc.vector.scalar_tensor_tensor(
                            out=scr[:, 512:1024], in0=e_t[:, 512:1024],
                            scalar=1.0 / TEMP, in1=ps[:, 512:1024],
                            op0=ALU.mult, op1=ALU.mult,
                            accum_out=qacc2,
                        )
                    else:
                        scr = scrpool.tile([128, JT], f32, tag="qv", name="scr")
                        nc.vector.scalar_tensor_tensor(
                            out=scr, in0=e_t, scalar=1.0 / TEMP,
                            in1=ps, op0=ALU.mult, op1=ALU.mult,
                            accum_out=qacc[:, rc, jt:jt + 1],
                        )

            # ---- combine (all DVE: cross-engine sem hops cost more than
            # the ~1.3us serial chain) ----
            p4 = stats.tile([128, RC], f32)
            nc.vector.reduce_sum(out=p4, in_=pacc, axis=AX.X)
            # u_new path (view-0 samples): un = g*em*P - g*ep + (1-g)u[idx]
            z2 = stats.tile([128, 2], f32)
            nc.vector.tensor_mul(z2, emg, p4[:, 0:2])
            un = stats.tile([128, 2], f32)
            nc.vector.tensor_add(un, z2, epu)
            ru = stats.tile([128, 2], f32)
            nc.vector.reciprocal(ru, un)

            q4 = stats.tile([128, RC], f32)
            nc.vector.reduce_sum(out=q4, in_=qacc, axis=AX.X)
            nc.vector.tensor_add(q4[:, RC - 1:RC], q4[:, RC - 1:RC], qacc2)
            t2 = stats.tile([128, RC], f32)
            nc.vector.tensor_mul(t2, m4, p4)
            t3 = stats.tile([128, RC], f32)
            nc.vector.tensor_sub(t3, q4, t2)
            w4 = stats.tile([128, RC], f32)
            nc.vector.tensor_mul(w4, em, t3)
            wn = stats.tile([128, RC], f32)
            nc.vector.tensor_sub(wn, w4, epl)
            c4 = stats.tile([128, RC], f32)
            nc.vector.tensor_mul(c4[:, 0:2], wn[:, 0:2], ru)
            nc.vector.tensor_mul(c4[:, 2:4], wn[:, 2:4], ru)
            out_sb = stats.tile([128, RC], f32)
            nc.vector.tensor_sub(out_sb, c4, lp2)
            nc.scalar.dma_start(out=out_d[:, :], in_=out_sb)

    nc.compile()
    return nc


def _get_module():
    if "nc" not in _CACHE:
        _CACHE["nc"] = _build_module()
    return _CACHE["nc"]


def _prep_inputs(index, features, u):
    feats = np.asarray(features, dtype=np.float32)
    idx = np.asarray(index).astype(np.int64).reshape(-1)
    u_np = np.asarray(u, dtype=np.float32).reshape(-1)

    cf = np.ascontiguousarray(feats.transpose(1, 0, 2).reshape(N, D))
    cf8 = cf.astype(ml_dtypes.float8_e4m3)
    ct8 = np.ascontiguousarray(cf8.T)                      # [D, N] fp8
    # [piece, 128, k0-block | k1-block]: piece i = columns [i*512,(i+1)*512)
    ct_in = np.ascontiguousarray(
        ct8.reshape(2, 128, N // 512, 512).transpose(2, 1, 0, 3)
        .reshape(N // 512, 128, 2 * 512))

    # per-row scalars from the f32 features (cheap O(N*D) host work)
    msum = np.einsum('nd,nd->n', cf, cf, dtype=np.float64).astype(np.float32)
    pdot = np.einsum('nd,nd->n', cf[:B], cf[B:],
                     dtype=np.float64).astype(np.float32)   # [B]
    m = msum / TEMP                                         # [N]
    pd4 = np.concatenate([pdot, pdot])                      # [N]
    lp = pd4 / TEMP - m                                     # Lpos [N]
    em = np.exp(-m)
    ep = np.exp(lp)
    epl = ep * lp

    in_maps = []
    for c in range(NCORES):
        rows = np.concatenate([
            np.arange(c * SPC, (c + 1) * SPC),
            np.arange(B + c * SPC, B + (c + 1) * SPC),
        ])
        anc_r = np.ascontiguousarray(ct8[:, rows])         # [128*2(k), RPC]
        anc = np.empty((128, 2 * RPC), dtype=ml_dtypes.float8_e4m3)
        anc[:, 0:RPC] = anc_r[0:128]
        anc[:, RPC:2 * RPC] = anc_r[128:256]

        # aux: [128, rc] layout matches the device's row chunks
        def rcview(v):
            return v[rows].reshape(RC, 128).T              # [128, RC]

        ug_vals = (1.0 - GAMMA) * u_np[idx[c * SPC:(c + 1) * SPC]]
        ug = ug_vals.reshape(2, 128).T                     # [128, 2]
        aux = np.empty((128, AUXW), dtype=np.float32)
        aux[:, 0:4] = rcview(m)
        aux[:, 4:8] = rcview(lp)
        aux[:, 8:12] = rcview(em)
        aux[:, 12:16] = rcview(epl)
        aux[:, 16:18] = GAMMA * rcview(em)[:, 0:2]         # emg (view0)
        aux[:, 18:20] = ug - GAMMA * rcview(ep)[:, 0:2]    # epu (view0)
        in_maps.append({"anc": anc, "aux": aux, "ct": ct_in})
    return in_maps


def _run(in_maps, trace=False, **kw):
    from concourse.bass_utils import run_bass_kernel_spmd

    nc = _get_module()
    return run_bass_kernel_spmd(
        nc, in_maps, core_ids=list(range(NCORES)), trace=trace, **kw
    )


def kernel(index, features, u):
    in_maps = _prep_inputs(index, features, u)
    res = _run(in_maps)
    total = 0.0
    for c in range(NCORES):
        total += np.asarray(res.results[c]["loss_rows"], dtype=np.float64).sum()
    return np.float32(total / N)
